# revision 14
# baseline (speedup 1.0000x reference)
"""GAT layer (N=8192, D=64) as a Bass/Tile kernel on 8 TRN2 NeuronCores.

Math (reference):
    h  = x @ W.T + b
    s1 = h @ a1 ; s2 = h @ a2                    # [N] each
    score[i,j] = s2[i] + s1[j]
    att = softmax_j(leaky_relu(score))
    out = att @ x

Reformulation:
    Fold the linear layer:  v = W.T @ [a1|a2], c_k = b.a_k
      p1 = x @ v1 ; p2 = x @ v2 ; s1 = p1 + c1 ; s2 = p2 + c2
    Softmax rows are shift invariant, so subtract p2[i] from row i. With
    per-j scalars E1 = exp(sh1), F1 = exp(0.01*sh1) (sh1 = p1 + c1 + c2)
    and the broadcast tile G2b[j,i] = exp(-0.99*p2[i]):
      e[j,i] = max( G2b[j,i] * F1[j],  E1[j] )
    The final matmul (ones-column appended to x for the softmax
    denominator) accumulates over j in PSUM:
      outT[0:65, i] += x_ext[j,:].T @ e[j,i] ; Z[i] = outT[64,i]

Schedule (informed by HW traces):
  - e-tiles are produced on DVE (58 tiles, tensor_scalar mult+max at the
    2x bf16 rate, ~584ns) and ACT (6 tiles as r = relu(E1 - F1*G2b); the
    missing rank-1 part F1[j]*G2[i] is restored by per-ACT-tile 1-column
    TF matmuls and one K=1 rank-1 matmul).  gpsimd must NOT touch the
    e-stream: its software tensor_scalar runs ~10x below roofline and it
    shares SBUF ports with DVE, dragging concurrent DVE ops to its speed.
  - The PE main matmuls pipeline at full rate (~209ns/512-col) when fed;
    the s1 projection runs as 64 tiny quadrant matmuls against xT
    (fp32r), ping-ponging two PSUM banks, mostly inside the pre-main DMA
    window.  ACT exps read the s1 PSUM directly.
  - Input DMAs are issued from the scalar/vector/gpsimd queues, which
    clear their framework preamble several us before the sync queue, and
    are sliced so every pipeline stage unblocks just-in-time.  The output
    is written back in 4 slices from 4 different queues as the epilogue
    drains.

Sharding: each core owns N/8 = 1024 query rows i. The host rotates the
j-order per core so the core's own block is always j 0..1023 (all cores
run the identical program; softmax sums are order-invariant).
"""

import sys
import types

import ml_dtypes
import numpy as np

import concourse.bacc as bacc
import concourse.bass as bass
import concourse.mybir as mybir
import concourse.tile as tile
from concourse.bass_utils import run_bass_kernel_spmd


def _install_ntff_hook_shim():
    """The agent image's ``antenv`` lacks ``axon_hooks``; provide it so
    ``run_bass_kernel_spmd(trace=True)`` can capture NTFF profiles."""
    if "antenv.axon_hooks" in sys.modules:
        return
    try:
        from trn_agent_boot.trn_boot import _ntff_profile_via_ctypes

        hook = _ntff_profile_via_ctypes("/opt/axon/libaxon_pjrt.so")
        mod = types.ModuleType("antenv.axon_hooks")
        mod._hook = hook
        mod.get_axon_ntff_profile_hook = lambda: mod._hook
        mod.set_axon_ntff_profile_hook = lambda h: setattr(mod, "_hook", h)
        sys.modules["antenv.axon_hooks"] = mod
    except Exception:
        pass


_install_ntff_hook_shim()

N, D = 8192, 64
NCORES = 8
RB = N // NCORES          # rows (i) per core = 1024
NT = N // 128             # j tiles of 128 = 64
BT = RB // 128            # i tiles per core = 8
DE = D + 1                # x extended with ones column = 65
F32 = mybir.dt.float32
F32R = mybir.dt.float32r
BF16 = mybir.dt.bfloat16
EXP = mybir.ActivationFunctionType.Exp
RELU = mybir.ActivationFunctionType.Relu
COPY = mybir.ActivationFunctionType.Copy
ADD = mybir.AluOpType.add
MUL = mybir.AluOpType.mult
MAX = mybir.AluOpType.max
AX_X = mybir.AxisListType.X
PKW = D + 3 + 128  # packed small-input width (W | b | a1 | a2 | ident)

# s1 chunk order follows input-slab arrival (chunk c = tiles 8c..8c+7):
# c0 from the xo sidecar, then (c4) slab0, (c1,c5) slab1, ...
CHUNKS = [0, 4, 1, 5, 2, 6, 3, 7]
JORDER = list(range(NT))  # mains consume tiles sequentially
# ACT takes 6 e-tiles (in the first 48 so TF closes early); DVE the rest.
_APOS = {6, 13, 20, 27, 34, 41}
EPAT = ["A" if n in _APOS else "V" for n in range(64)]


def build_bass() -> bass.Bass:
    nc = bacc.Bacc(None)
    # xq: x.T quadrant-packed fp32: partitions 0:64 = d rows for j 0..4095
    # (cols = j); partitions 64:128 = d rows for j 4096..8191.
    xq_d = nc.declare_dram_parameter("xq", [128, 4096], F32R, isOutput=False)
    # xo: own-block sidecar: partitions 0:64 = d x j 0..511, 64:128 = d x
    # j 512..1023 (gates G2b + s1 chunk 0 well before the big xq slabs).
    xo_d = nc.declare_dram_parameter("xo", [128, 512], F32R, isOutput=False)
    # xbf: bf16 (x | ones) partition-major: [128, t, de]
    xbf_d = nc.declare_dram_parameter("xbf", [128, NT * DE], BF16, isOutput=False)
    pk_d = nc.declare_dram_parameter("pack", [128, PKW], F32, isOutput=False)
    out_d = nc.declare_dram_parameter("out", [128, BT * D], F32, isOutput=True)

    with tile.TileContext(nc) as tc:
        with (
            tc.tile_pool(name="persist", bufs=1) as persist,
            tc.tile_pool(name="small", bufs=1) as small,
            tc.tile_pool(name="epool", bufs=12) as epool,
            tc.tile_pool(name="opool", bufs=2) as opool,
            tc.tile_pool(name="psumA", bufs=3, space="PSUM") as psumA,
            tc.tile_pool(name="psumS", bufs=1, space="PSUM") as psumS,
            tc.tile_pool(name="psumB", bufs=1, space="PSUM") as psumB,
        ):
            pk = small.tile([128, PKW], F32)
            xo = small.tile([128, 512], F32R)
            xq = persist.tile([128, 4096], F32R)
            xbf_flat = persist.tile([128, NT * DE], BF16)
            x_bf = xbf_flat.rearrange("p (t d) -> p t d", t=NT)

            # ---- input DMAs, spread across early-clearing queues ----
            nc.scalar.dma_start(pk, pk_d[:, :])
            nc.scalar.dma_start(xo, xo_d[:, :])
            nc.scalar.dma_start(xbf_flat[:, 0 : 16 * DE], xbf_d[:, 0 : 16 * DE])
            nc.gpsimd.dma_start(xq[:, 0:1024], xq_d[:, 0:1024])
            nc.gpsimd.dma_start(xq[:, 1024:2048], xq_d[:, 1024:2048])
            nc.gpsimd.dma_start(
                xbf_flat[:, 16 * DE : 32 * DE], xbf_d[:, 16 * DE : 32 * DE]
            )
            nc.sync.dma_start(xq[:, 2048:3072], xq_d[:, 2048:3072])
            nc.sync.dma_start(
                xbf_flat[:, 32 * DE : 48 * DE], xbf_d[:, 32 * DE : 48 * DE]
            )
            nc.sync.dma_start(xq[:, 3072:4096], xq_d[:, 3072:4096])
            nc.sync.dma_start(
                xbf_flat[:, 48 * DE : 64 * DE], xbf_d[:, 48 * DE : 64 * DE]
            )

            W_lo = pk[0:D, 0:D]
            b_lo = pk[0:D, D : D + 1]
            a_lo = pk[0:D, D + 1 : D + 3]
            W_hi = pk[64:128, 0:D]
            a_hi = pk[64:128, D + 1 : D + 3]
            ident = pk[:, D + 3 : D + 3 + 128]

            ones_row = small.tile([1, 128], F32)
            nc.gpsimd.memset(ones_row, 1.0)
            ones_bf = small.tile([1, 128], BF16)
            nc.gpsimd.memset(ones_bf, 1.0)
            # F1 interleaved with zeros (bf16) so TF matmuls have free=2
            F1cb2 = small.tile([128, NT, 2], BF16)
            nc.gpsimd.memset(F1cb2, 0.0)

            # ---------------- tiny projections on PE ----------------
            # v = W.T @ [a1|a2], duplicated into both partition halves so
            # upper-quadrant matmuls have a local rhs.
            v_ps = psumA.tile([128, 2], F32, tag="ps", name="v_ps")
            nc.tensor.matmul(v_ps[0:64, :], lhsT=W_lo, rhs=a_lo, start=True, stop=True)
            nc.tensor.matmul(v_ps[64:128, :], lhsT=W_hi, rhs=a_hi, start=True, stop=True)
            v_r = small.tile([128, 2], F32R)
            nc.scalar.copy(out=v_r, in_=v_ps)

            # c = [b.a1, b.a2] ; c12 = c1+c2 broadcast down 128 partitions
            c_ps = psumA.tile([1, 2], F32, tag="ps", name="c_ps")
            nc.tensor.matmul(c_ps, lhsT=b_lo, rhs=a_lo, start=True, stop=True)
            c_sb = small.tile([1, 2], F32)
            nc.scalar.copy(out=c_sb, in_=c_ps)
            cb_ps = psumA.tile([128, 2], F32, tag="ps", name="cb_ps")
            nc.tensor.matmul(cb_ps, lhsT=ones_row, rhs=c_sb, start=True, stop=True)
            c12 = small.tile([128, 1], F32)
            nc.vector.tensor_reduce(out=c12, in_=cb_ps, axis=AX_X, op=ADD)
            c12s = small.tile([128, 1], F32)
            nc.vector.tensor_scalar(
                out=c12s, in0=c12, scalar1=0.01, scalar2=None, op0=MUL
            )

            # ---------------- p2 of own block -> G2row, G2b ----------------
            G2b = persist.tile([128, RB], BF16)
            g2rows = []
            for h in range(2):
                p2r_ps = psumA.tile([1, 512], F32, tag="ps", name="p2r_ps")
                nc.tensor.matmul(
                    p2r_ps,
                    lhsT=v_r[64 * h : 64 * h + 64, 1:2],
                    rhs=xo[64 * h : 64 * h + 64, :],
                    start=True,
                    stop=True,
                )
                g2row = small.tile([1, 512], BF16, tag=f"g2row{h}", name=f"g2row{h}")
                nc.scalar.activation(out=g2row, in_=p2r_ps, func=EXP, scale=-0.99)
                g2rows.append(g2row)
                gb_ps = psumA.tile([128, 512], F32, tag="ps", name="gb_ps")
                nc.tensor.matmul(
                    gb_ps, lhsT=ones_bf, rhs=g2row, start=True, stop=True
                )
                nc.scalar.copy(out=G2b[:, h * 512 : (h + 1) * 512], in_=gb_ps)

            # ---------------- s1 on PE + exps on ACT ----------------
            # s1 tile t: lhsT = [64,128] quadrant slice of xT, rhs = v
            # -> psum [128, 2] (col 0 = p1).  Within a chunk the 8 matmuls
            # ping-pong two PSUM banks so they pipeline; ACT exps read the
            # PSUM halves directly (strided writes into E1c/F1c).
            E1c = small.tile([128, NT], F32)
            F1c = small.tile([128, NT], F32)
            nF1c = small.tile([128, NT], F32)
            for c in CHUNKS:
                halves = [
                    psumS.tile([128, 4, 2], F32, tag="s1a", name=f"s1a{c}"),
                    psumS.tile([128, 4, 2], F32, tag="s1b", name=f"s1b{c}"),
                ]
                for k in range(8):
                    t = 8 * c + k
                    if c == 0:
                        lhsT = xo[64 * (t // 4) : 64 * (t // 4) + 64,
                                  128 * (t % 4) : 128 * (t % 4) + 128]
                        rhs = v_r[64 * (t // 4) : 64 * (t // 4) + 64, 0:2]
                    elif t < 32:
                        lhsT = xq[0:64, 128 * t : 128 * (t + 1)]
                        rhs = v_r[0:64, 0:2]
                    else:
                        lhsT = xq[64:128, 128 * (t - 32) : 128 * (t - 31)]
                        rhs = v_r[64:128, 0:2]
                    nc.tensor.matmul(
                        halves[k % 2][:, k // 2, :],
                        lhsT=lhsT,
                        rhs=rhs,
                        start=True,
                        stop=True,
                        skip_group_check=True,
                    )
                for par in range(2):
                    cs = slice(8 * c + par, 8 * c + 8, 2)
                    p1col = halves[par][:, :, 0:1]
                    nc.scalar.activation(
                        out=E1c[:, cs], in_=p1col, func=EXP, bias=c12, scale=1.0
                    )
                    nc.scalar.activation(
                        out=F1c[:, cs], in_=p1col, func=EXP, bias=c12s, scale=0.01
                    )
                cs = slice(8 * c, 8 * (c + 1))
                nc.scalar.copy(out=F1cb2[:, cs, 0:1], in_=F1c[:, cs])
                nc.vector.tensor_scalar(
                    out=nF1c[:, cs], in0=F1c[:, cs], scalar1=-1.0, scalar2=None,
                    op0=MUL,
                )

            # ---------------- main loop ----------------
            acc0 = psumB.tile([128, 512], F32, tag="acc0", name="acc0")
            acc1 = psumB.tile([128, 512], F32, tag="acc1", name="acc1")
            accs = [acc0, acc1]
            tf_ps = psumB.tile([128, 2], F32, tag="tf", name="tf_ps")
            act_jts = [jt for n, jt in enumerate(JORDER) if EPAT[n] == "A"]
            first_act, last_act = act_jts[0], act_jts[-1]

            for n, jt in enumerate(JORDER):
                eng = EPAT[n]
                e_t = epool.tile([128, RB], BF16, tag="e", name="e_t")
                if eng == "V":
                    nc.vector.tensor_scalar(
                        out=e_t, in0=G2b,
                        scalar1=F1c[:, jt : jt + 1], scalar2=E1c[:, jt : jt + 1],
                        op0=MUL, op1=MAX,
                    )
                else:  # ACT: relu(E1 - F1*G2b); rank-1 part restored later
                    nc.scalar.activation(
                        out=e_t, in_=G2b, func=RELU,
                        scale=nF1c[:, jt : jt + 1], bias=E1c[:, jt : jt + 1],
                    )
                lhsT = x_bf[:, jt, 0:DE]
                for h in range(2):
                    nc.tensor.matmul(
                        accs[h][0:DE, :],
                        lhsT=lhsT,
                        rhs=e_t[:, h * 512 : (h + 1) * 512],
                        start=(n == 0),
                        stop=False,
                    )
                if eng == "A":
                    nc.tensor.matmul(
                        tf_ps[0:DE, :],
                        lhsT=lhsT,
                        rhs=F1cb2[:, jt, :],
                        start=(jt == first_act),
                        stop=(jt == last_act),
                    )

            # rank-1 completion: accs[h] += TF_act[d] * G2row[i]
            tf_sb = small.tile([DE, 1], F32)
            nc.scalar.copy(out=tf_sb, in_=tf_ps[0:DE, 0:1])
            tfT_ps = psumA.tile([1, DE], F32, tag="ps", name="tfT_ps")
            nc.tensor.transpose(tfT_ps, tf_sb, ident[:DE, :DE])
            tf_row = small.tile([1, DE], BF16)
            nc.scalar.copy(out=tf_row, in_=tfT_ps)
            for h in range(2):
                nc.tensor.matmul(
                    accs[h][0:DE, :],
                    lhsT=tf_row,
                    rhs=g2rows[h],
                    start=False,
                    stop=True,
                )

            # ---------------- epilogue: normalize + store ----------------
            # per t-tile: PE transpose -> DVE reciprocal -> ACT normalize;
            # output leaves in 4 slices from 4 different DMA queues.
            outT = small.tile([DE, RB], F32)
            for h in range(2):
                nc.scalar.copy(
                    out=outT[:, h * 512 : (h + 1) * 512],
                    in_=accs[h][0:DE, :],
                )
            out_flat = small.tile([128, BT * D], F32)
            out_sb = out_flat.rearrange("p (t d) -> p t d", t=BT)
            oq = [nc.sync, nc.scalar, nc.gpsimd, nc.scalar]
            for t in range(BT):
                tp2 = psumA.tile([128, DE], F32, tag="ps", name="tp2")
                nc.tensor.transpose(
                    tp2, outT[:, t * 128 : (t + 1) * 128], ident[:DE, :DE]
                )
                rcol = opool.tile([128, 1], F32, tag="rcol", name="rcol")
                nc.vector.reciprocal(rcol, tp2[:, D : D + 1])
                nc.scalar.activation(
                    out=out_sb[:, t, :], in_=tp2[:, 0:D], func=COPY, scale=rcol
                )
                if t % 2 == 1:
                    q = oq[t // 2]
                    q.dma_start(
                        out_d[:, (t - 1) * D : (t + 1) * D],
                        out_flat[:, (t - 1) * D : (t + 1) * D],
                    )

    nc.finalize()
    return nc


def _execute(inputs: dict, trace: bool = False):
    x = np.ascontiguousarray(np.asarray(inputs["x"], dtype=np.float32))
    W = np.ascontiguousarray(np.asarray(inputs["W"], dtype=np.float32))
    b = np.asarray(inputs["b"], dtype=np.float32).reshape(D)
    a = np.asarray(inputs["a"], dtype=np.float32).reshape(2 * D)
    assert x.shape == (N, D) and W.shape == (D, D)

    pack0 = np.zeros((128, PKW), np.float32)
    for half in (slice(0, 64), slice(64, 128)):
        pack0[half, 0:D] = W
        pack0[half, D] = b
        pack0[half, D + 1] = a[:D]
        pack0[half, D + 2] = a[D:]
    pack0[:, D + 3 : D + 3 + 128] = np.eye(128, dtype=np.float32)

    xe = np.concatenate([x, np.ones((N, 1), np.float32)], axis=1)  # [N, 65]

    nc = build_bass()
    in_maps = []
    for c in range(NCORES):
        # rotate j-order so this core's own block is j 0..1023
        perm = np.concatenate(
            [np.arange(c * RB, (c + 1) * RB), np.arange(0, c * RB),
             np.arange((c + 1) * RB, N)]
        )
        xT = x[perm].T          # [D, N] rotated
        xq = np.ascontiguousarray(
            np.concatenate([xT[:, 0:4096], xT[:, 4096:8192]], axis=0)
        )
        xo = np.ascontiguousarray(
            np.concatenate([xT[:, 0:512], xT[:, 512:1024]], axis=0)
        )
        xbf = np.ascontiguousarray(
            xe[perm].reshape(NT, 128, DE)
            .transpose(1, 0, 2)
            .reshape(128, NT * DE)
            .astype(ml_dtypes.bfloat16)
        )
        in_maps.append({"xq": xq, "xo": xo, "xbf": xbf, "pack": pack0})
    res = run_bass_kernel_spmd(
        nc, in_maps, core_ids=list(range(NCORES)), trace=trace
    )
    # un-permute each core's output: (p, t*D+d) -> (t*128+p, d)
    outs = []
    for r in res.results:
        o = r["out"].reshape(128, BT, D).transpose(1, 0, 2).reshape(RB, D)
        outs.append(o)
    out = np.ascontiguousarray(np.concatenate(outs, axis=0))
    return out, res


def kernel(x, W, b, a):
    out, _ = _execute({"x": x, "W": W, "b": b, "a": a})
    return out


# revision 18
# speedup vs baseline: 1.0910x; 1.0910x over previous
"""GAT layer (N=8192, D=64) as a Bass/Tile kernel on 8 TRN2 NeuronCores.

Math (reference):
    h  = x @ W.T + b
    s1 = h @ a1 ; s2 = h @ a2                    # [N] each
    score[i,j] = s2[i] + s1[j]
    att = softmax_j(leaky_relu(score))
    out = att @ x

Reformulation:
    Fold the linear layer:  v = W.T @ [a1|a2], c_k = b.a_k
      p1 = x @ v1 ; p2 = x @ v2 ; s1 = p1 + c1 ; s2 = p2 + c2
    Softmax rows are shift invariant, so subtract p2[i] from row i. With
    per-j scalars E1 = exp(sh1), F1 = exp(0.01*sh1) (sh1 = p1 + c1 + c2)
    and the broadcast tile G2b[j,i] = exp(-0.99*p2[i]):
      e[j,i] = max( G2b[j,i] * F1[j],  E1[j] )
    The final matmul (ones-column appended to x for the softmax
    denominator) accumulates over j in PSUM:
      outT[0:65, i] += x_ext[j,:].T @ e[j,i] ; Z[i] = outT[64,i]

Schedule (informed by HW traces):
  - e-tiles are produced on DVE (58 tiles, tensor_scalar mult+max at the
    2x bf16 rate, ~584ns) and ACT (6 tiles as r = relu(E1 - F1*G2b); the
    missing rank-1 part F1[j]*G2[i] is restored by per-ACT-tile 1-column
    TF matmuls and one K=1 rank-1 matmul).  gpsimd must NOT touch the
    e-stream: its software tensor_scalar runs ~10x below roofline and it
    shares SBUF ports with DVE, dragging concurrent DVE ops to its speed.
  - The PE main matmuls pipeline at full rate (~209ns/512-col) when fed;
    the s1 projection runs as 64 tiny quadrant matmuls against xT
    (fp32r), ping-ponging two PSUM banks, mostly inside the pre-main DMA
    window.  ACT exps read the s1 PSUM directly.
  - Input DMAs are issued from the scalar/vector/gpsimd queues, which
    clear their framework preamble several us before the sync queue, and
    are sliced so every pipeline stage unblocks just-in-time.  The output
    is written back in 4 slices from 4 different queues as the epilogue
    drains.

Sharding: each core owns N/8 = 1024 query rows i. The host rotates the
j-order per core so the core's own block is always j 0..1023 (all cores
run the identical program; softmax sums are order-invariant).
"""

import sys
import types

import ml_dtypes
import numpy as np

import concourse.bacc as bacc
import concourse.bass as bass
import concourse.mybir as mybir
import concourse.tile as tile
from concourse.bass_utils import run_bass_kernel_spmd


def _install_ntff_hook_shim():
    """The agent image's ``antenv`` lacks ``axon_hooks``; provide it so
    ``run_bass_kernel_spmd(trace=True)`` can capture NTFF profiles."""
    if "antenv.axon_hooks" in sys.modules:
        return
    try:
        from trn_agent_boot.trn_boot import _ntff_profile_via_ctypes

        hook = _ntff_profile_via_ctypes("/opt/axon/libaxon_pjrt.so")
        mod = types.ModuleType("antenv.axon_hooks")
        mod._hook = hook
        mod.get_axon_ntff_profile_hook = lambda: mod._hook
        mod.set_axon_ntff_profile_hook = lambda h: setattr(mod, "_hook", h)
        sys.modules["antenv.axon_hooks"] = mod
    except Exception:
        pass


_install_ntff_hook_shim()

N, D = 8192, 64
NCORES = 8
RB = N // NCORES          # rows (i) per core = 1024
NT = N // 128             # j tiles of 128 = 64
BT = RB // 128            # i tiles per core = 8
DE = D + 1                # x extended with ones column = 65
F32 = mybir.dt.float32
F32R = mybir.dt.float32r
BF16 = mybir.dt.bfloat16
EXP = mybir.ActivationFunctionType.Exp
RELU = mybir.ActivationFunctionType.Relu
COPY = mybir.ActivationFunctionType.Copy
ADD = mybir.AluOpType.add
MUL = mybir.AluOpType.mult
MAX = mybir.AluOpType.max
AX_X = mybir.AxisListType.X
PKW = D + 3 + 128  # packed small-input width (W | b | a1 | a2 | ident)

# s1 chunk order follows input-slab arrival (chunk c = tiles 8c..8c+7):
# c0 from the xo sidecar, then (c4) slab0, (c1,c5) slab1, ...
CHUNKS = [0, 4, 1, 5, 2, 6, 3, 7]
JORDER = list(range(NT))  # mains consume tiles sequentially
# ACT takes 6 e-tiles (in the first 48 so TF closes early); DVE the rest.
_APOS = {6, 13, 20, 27, 34, 41}
EPAT = ["A" if n in _APOS else "V" for n in range(64)]


def build_bass() -> bass.Bass:
    nc = bacc.Bacc(None)
    # xq: x.T quadrant-packed fp32: partitions 0:64 = d rows for j 0..4095
    # (cols = j); partitions 64:128 = d rows for j 4096..8191.
    xq_d = nc.declare_dram_parameter("xq", [128, 4096], F32R, isOutput=False)
    # xo: own-block sidecar: partitions 0:64 = d x j 0..511, 64:128 = d x
    # j 512..1023 (gates G2b + s1 chunk 0 well before the big xq slabs).
    xo_d = nc.declare_dram_parameter("xo", [128, 512], F32R, isOutput=False)
    # xbf: bf16 (x | ones) partition-major: [128, t, de]
    xbf_d = nc.declare_dram_parameter("xbf", [128, NT * DE], BF16, isOutput=False)
    pk_d = nc.declare_dram_parameter("pack", [128, PKW], F32, isOutput=False)
    out_d = nc.declare_dram_parameter("out", [128, BT * D], F32, isOutput=True)

    with tile.TileContext(nc) as tc:
        with (
            tc.tile_pool(name="persist", bufs=1) as persist,
            tc.tile_pool(name="small", bufs=1) as small,
            tc.tile_pool(name="epool", bufs=12) as epool,
            tc.tile_pool(name="opool", bufs=2) as opool,
            tc.tile_pool(name="psumA", bufs=3, space="PSUM") as psumA,
            tc.tile_pool(name="psumS", bufs=1, space="PSUM") as psumS,
            tc.tile_pool(name="psumB", bufs=1, space="PSUM") as psumB,
        ):
            pk = small.tile([128, PKW], F32)
            xo = small.tile([128, 512], F32R)
            xq = persist.tile([128, 4096], F32R)
            xbf_flat = persist.tile([128, NT * DE], BF16)
            x_bf = xbf_flat.rearrange("p (t d) -> p t d", t=NT)

            # ---- input DMAs ----
            # scalar's queue clears its preamble first: give it ONLY the
            # two tiny prologue-gating inputs so the ACT chain (g2row,
            # exps) isn't stuck behind DMA descriptor generation.  The
            # bulk slabs ride on gpsimd (SWDGE, idle engine) and sync.
            nc.scalar.dma_start(pk, pk_d[:, :])
            nc.scalar.dma_start(xo, xo_d[:, :])
            nc.gpsimd.dma_start(xq[:, 0:1024], xq_d[:, 0:1024])
            nc.gpsimd.dma_start(xq[:, 1024:2048], xq_d[:, 1024:2048])
            nc.sync.dma_start(xbf_flat[:, 0 : 16 * DE], xbf_d[:, 0 : 16 * DE])
            nc.sync.dma_start(
                xbf_flat[:, 16 * DE : 32 * DE], xbf_d[:, 16 * DE : 32 * DE]
            )
            nc.sync.dma_start(xq[:, 2048:3072], xq_d[:, 2048:3072])
            nc.sync.dma_start(
                xbf_flat[:, 32 * DE : 48 * DE], xbf_d[:, 32 * DE : 48 * DE]
            )
            nc.sync.dma_start(xq[:, 3072:4096], xq_d[:, 3072:4096])
            nc.sync.dma_start(
                xbf_flat[:, 48 * DE : 64 * DE], xbf_d[:, 48 * DE : 64 * DE]
            )

            W_lo = pk[0:D, 0:D]
            b_lo = pk[0:D, D : D + 1]
            a_lo = pk[0:D, D + 1 : D + 3]
            W_hi = pk[64:128, 0:D]
            a_hi = pk[64:128, D + 1 : D + 3]
            ident = pk[:, D + 3 : D + 3 + 128]

            ones_row = small.tile([1, 128], F32)
            nc.gpsimd.memset(ones_row, 1.0)
            ones_bf = small.tile([1, 128], BF16)
            nc.gpsimd.memset(ones_bf, 1.0)
            # F1 interleaved with zeros (bf16) so TF matmuls have free=2
            F1cb2 = small.tile([128, NT, 2], BF16)
            nc.gpsimd.memset(F1cb2, 0.0)

            # ---------------- tiny projections on PE ----------------
            # v = W.T @ [a1|a2], duplicated into both partition halves so
            # upper-quadrant matmuls have a local rhs.
            v_ps = psumA.tile([128, 2], F32, tag="ps", name="v_ps")
            nc.tensor.matmul(v_ps[0:64, :], lhsT=W_lo, rhs=a_lo, start=True, stop=True)
            nc.tensor.matmul(v_ps[64:128, :], lhsT=W_hi, rhs=a_hi, start=True, stop=True)
            v_r = small.tile([128, 2], F32R)
            nc.vector.tensor_copy(out=v_r, in_=v_ps)

            # c = [b.a1, b.a2] ; c12 = c1+c2 broadcast down 128 partitions
            c_ps = psumA.tile([1, 2], F32, tag="ps", name="c_ps")
            nc.tensor.matmul(c_ps, lhsT=b_lo, rhs=a_lo, start=True, stop=True)
            c_sb = small.tile([1, 2], F32)
            nc.vector.tensor_copy(out=c_sb, in_=c_ps)
            cb_ps = psumA.tile([128, 2], F32, tag="ps", name="cb_ps")
            nc.tensor.matmul(cb_ps, lhsT=ones_row, rhs=c_sb, start=True, stop=True)
            c12 = small.tile([128, 1], F32)
            nc.vector.tensor_reduce(out=c12, in_=cb_ps, axis=AX_X, op=ADD)
            c12s = small.tile([128, 1], F32)
            nc.vector.tensor_scalar(
                out=c12s, in0=c12, scalar1=0.01, scalar2=None, op0=MUL
            )

            # ---------------- p2 of own block -> G2row, G2b ----------------
            G2b = persist.tile([128, RB], BF16)
            g2rows = []
            for h in range(2):
                p2r_ps = psumA.tile([1, 512], F32, tag="ps", name="p2r_ps")
                nc.tensor.matmul(
                    p2r_ps,
                    lhsT=v_r[64 * h : 64 * h + 64, 1:2],
                    rhs=xo[64 * h : 64 * h + 64, :],
                    start=True,
                    stop=True,
                )
                g2row = small.tile([1, 512], BF16, tag=f"g2row{h}", name=f"g2row{h}")
                nc.scalar.activation(out=g2row, in_=p2r_ps, func=EXP, scale=-0.99)
                g2rows.append(g2row)
                gb_ps = psumA.tile([128, 512], F32, tag="ps", name="gb_ps")
                nc.tensor.matmul(
                    gb_ps, lhsT=ones_bf, rhs=g2row, start=True, stop=True
                )
                nc.vector.tensor_copy(
                    out=G2b[:, h * 512 : (h + 1) * 512], in_=gb_ps
                )

            # ---------------- s1 on PE + exps on ACT ----------------
            # s1 tile t: lhsT = [64,128] quadrant slice of xT, rhs = v
            # -> psum [128, 2] (col 0 = p1).  Within a chunk the 8 matmuls
            # ping-pong two PSUM banks so they pipeline; ACT exps read the
            # PSUM halves directly (strided writes into E1c/F1c).
            E1c = small.tile([128, NT], F32)
            F1c = small.tile([128, NT], F32)
            nF1c = small.tile([128, NT], F32)
            for c in CHUNKS:
                halves = [
                    psumS.tile([128, 4, 2], F32, tag="s1a", name=f"s1a{c}"),
                    psumS.tile([128, 4, 2], F32, tag="s1b", name=f"s1b{c}"),
                ]
                for k in range(8):
                    t = 8 * c + k
                    if c == 0:
                        lhsT = xo[64 * (t // 4) : 64 * (t // 4) + 64,
                                  128 * (t % 4) : 128 * (t % 4) + 128]
                        rhs = v_r[64 * (t // 4) : 64 * (t // 4) + 64, 0:2]
                    elif t < 32:
                        lhsT = xq[0:64, 128 * t : 128 * (t + 1)]
                        rhs = v_r[0:64, 0:2]
                    else:
                        lhsT = xq[64:128, 128 * (t - 32) : 128 * (t - 31)]
                        rhs = v_r[64:128, 0:2]
                    nc.tensor.matmul(
                        halves[k % 2][:, k // 2, :],
                        lhsT=lhsT,
                        rhs=rhs,
                        start=True,
                        stop=True,
                        skip_group_check=True,
                    )
                for par in range(2):
                    cs = slice(8 * c + par, 8 * c + 8, 2)
                    p1col = halves[par][:, :, 0:1]
                    nc.scalar.activation(
                        out=E1c[:, cs], in_=p1col, func=EXP, bias=c12, scale=1.0
                    )
                    nc.scalar.activation(
                        out=F1c[:, cs], in_=p1col, func=EXP, bias=c12s, scale=0.01
                    )
                cs = slice(8 * c, 8 * (c + 1))
                nc.scalar.copy(out=F1cb2[:, cs, 0:1], in_=F1c[:, cs])
                nc.vector.tensor_scalar(
                    out=nF1c[:, cs], in0=F1c[:, cs], scalar1=-1.0, scalar2=None,
                    op0=MUL,
                )

            # perf probes in the pre-main DVE idle window: measure whether
            # 1-ptr-scalar and immediate-scalar tensor_scalar variants
            # reach the 4x DVE rate (the 2-ptr mult+max op measures 2x).
            # Results are read from the trace; outputs are unused scratch.
            probe = small.tile([128, RB], BF16)
            nc.vector.tensor_scalar(
                out=probe, in0=G2b, scalar1=E1c[:, 0:1], scalar2=None, op0=MAX
            )
            nc.vector.tensor_scalar(
                out=probe, in0=G2b, scalar1=2.0, scalar2=None, op0=MUL
            )

            # ---------------- main loop ----------------
            acc0 = psumB.tile([128, 512], F32, tag="acc0", name="acc0")
            acc1 = psumB.tile([128, 512], F32, tag="acc1", name="acc1")
            accs = [acc0, acc1]
            tf_ps = psumB.tile([128, 2], F32, tag="tf", name="tf_ps")
            act_jts = [jt for n, jt in enumerate(JORDER) if EPAT[n] == "A"]
            first_act, last_act = act_jts[0], act_jts[-1]

            for n, jt in enumerate(JORDER):
                eng = EPAT[n]
                e_t = epool.tile([128, RB], BF16, tag="e", name="e_t")
                if eng == "V":
                    nc.vector.tensor_scalar(
                        out=e_t, in0=G2b,
                        scalar1=F1c[:, jt : jt + 1], scalar2=E1c[:, jt : jt + 1],
                        op0=MUL, op1=MAX,
                    )
                else:  # ACT: relu(E1 - F1*G2b); rank-1 part restored later
                    nc.scalar.activation(
                        out=e_t, in_=G2b, func=RELU,
                        scale=nF1c[:, jt : jt + 1], bias=E1c[:, jt : jt + 1],
                    )
                lhsT = x_bf[:, jt, 0:DE]
                for h in range(2):
                    nc.tensor.matmul(
                        accs[h][0:DE, :],
                        lhsT=lhsT,
                        rhs=e_t[:, h * 512 : (h + 1) * 512],
                        start=(n == 0),
                        stop=False,
                    )
                if eng == "A":
                    nc.tensor.matmul(
                        tf_ps[0:DE, :],
                        lhsT=lhsT,
                        rhs=F1cb2[:, jt, :],
                        start=(jt == first_act),
                        stop=(jt == last_act),
                    )

            # rank-1 completion: accs[h] += TF_act[d] * G2row[i]
            tf_sb = small.tile([DE, 1], F32)
            nc.scalar.copy(out=tf_sb, in_=tf_ps[0:DE, 0:1])
            tfT_ps = psumA.tile([1, DE], F32, tag="ps", name="tfT_ps")
            nc.tensor.transpose(tfT_ps, tf_sb, ident[:DE, :DE])
            tf_row = small.tile([1, DE], BF16)
            nc.scalar.copy(out=tf_row, in_=tfT_ps)
            for h in range(2):
                nc.tensor.matmul(
                    accs[h][0:DE, :],
                    lhsT=tf_row,
                    rhs=g2rows[h],
                    start=False,
                    stop=True,
                )

            # ---------------- epilogue: normalize + store ----------------
            # per t-tile: PE transpose -> DVE reciprocal -> ACT normalize;
            # output leaves in 4 slices from 4 different DMA queues.
            outT = small.tile([DE, RB], F32)
            for h in range(2):
                nc.scalar.copy(
                    out=outT[:, h * 512 : (h + 1) * 512],
                    in_=accs[h][0:DE, :],
                )
            out_flat = small.tile([128, BT * D], F32)
            out_sb = out_flat.rearrange("p (t d) -> p t d", t=BT)
            oq = [nc.sync, nc.scalar, nc.gpsimd, nc.scalar]
            for t in range(BT):
                tp2 = psumA.tile([128, DE], F32, tag="ps", name="tp2")
                nc.tensor.transpose(
                    tp2, outT[:, t * 128 : (t + 1) * 128], ident[:DE, :DE]
                )
                rcol = opool.tile([128, 1], F32, tag="rcol", name="rcol")
                nc.vector.reciprocal(rcol, tp2[:, D : D + 1])
                nc.scalar.activation(
                    out=out_sb[:, t, :], in_=tp2[:, 0:D], func=COPY, scale=rcol
                )
                if t % 2 == 1:
                    q = oq[t // 2]
                    q.dma_start(
                        out_d[:, (t - 1) * D : (t + 1) * D],
                        out_flat[:, (t - 1) * D : (t + 1) * D],
                    )

    nc.finalize()
    return nc


def _execute(inputs: dict, trace: bool = False):
    x = np.ascontiguousarray(np.asarray(inputs["x"], dtype=np.float32))
    W = np.ascontiguousarray(np.asarray(inputs["W"], dtype=np.float32))
    b = np.asarray(inputs["b"], dtype=np.float32).reshape(D)
    a = np.asarray(inputs["a"], dtype=np.float32).reshape(2 * D)
    assert x.shape == (N, D) and W.shape == (D, D)

    pack0 = np.zeros((128, PKW), np.float32)
    for half in (slice(0, 64), slice(64, 128)):
        pack0[half, 0:D] = W
        pack0[half, D] = b
        pack0[half, D + 1] = a[:D]
        pack0[half, D + 2] = a[D:]
    pack0[:, D + 3 : D + 3 + 128] = np.eye(128, dtype=np.float32)

    xe = np.concatenate([x, np.ones((N, 1), np.float32)], axis=1)  # [N, 65]

    nc = build_bass()
    in_maps = []
    for c in range(NCORES):
        # rotate j-order so this core's own block is j 0..1023
        perm = np.concatenate(
            [np.arange(c * RB, (c + 1) * RB), np.arange(0, c * RB),
             np.arange((c + 1) * RB, N)]
        )
        xT = x[perm].T          # [D, N] rotated
        xq = np.ascontiguousarray(
            np.concatenate([xT[:, 0:4096], xT[:, 4096:8192]], axis=0)
        )
        xo = np.ascontiguousarray(
            np.concatenate([xT[:, 0:512], xT[:, 512:1024]], axis=0)
        )
        xbf = np.ascontiguousarray(
            xe[perm].reshape(NT, 128, DE)
            .transpose(1, 0, 2)
            .reshape(128, NT * DE)
            .astype(ml_dtypes.bfloat16)
        )
        in_maps.append({"xq": xq, "xo": xo, "xbf": xbf, "pack": pack0})
    res = run_bass_kernel_spmd(
        nc, in_maps, core_ids=list(range(NCORES)), trace=trace
    )
    # un-permute each core's output: (p, t*D+d) -> (t*128+p, d)
    outs = []
    for r in res.results:
        o = r["out"].reshape(128, BT, D).transpose(1, 0, 2).reshape(RB, D)
        outs.append(o)
    out = np.ascontiguousarray(np.concatenate(outs, axis=0))
    return out, res


def kernel(x, W, b, a):
    out, _ = _execute({"x": x, "W": W, "b": b, "a": a})
    return out


# revision 21
# speedup vs baseline: 1.1454x; 1.0499x over previous
"""GAT layer (N=8192, D=64) as a Bass/Tile kernel on 8 TRN2 NeuronCores.

Math (reference):
    h  = x @ W.T + b
    s1 = h @ a1 ; s2 = h @ a2                    # [N] each
    score[i,j] = s2[i] + s1[j]
    att = softmax_j(leaky_relu(score))
    out = att @ x

Reformulation:
    Fold the linear layer:  v = W.T @ [a1|a2], c_k = b.a_k
      p1 = x @ v1 ; p2 = x @ v2 ; s1 = p1 + c1 ; s2 = p2 + c2
    Softmax rows are shift invariant, so subtract p2[i] from row i. With
    per-j scalars E1 = exp(sh1), F1 = exp(0.01*sh1) (sh1 = p1 + c1 + c2)
    and the broadcast tile G2b[j,i] = exp(-0.99*p2[i]):
      e[j,i] = max( G2b[j,i] * F1[j],  E1[j] )
    The final matmul (ones-column appended to x for the softmax
    denominator) accumulates over j in PSUM:
      outT[0:65, i] += x_ext[j,:].T @ e[j,i] ; Z[i] = outT[64,i]

Schedule (informed by HW traces):
  - e-tiles are produced on DVE (58 tiles, tensor_scalar mult+max at the
    2x bf16 rate, ~584ns) and ACT (6 tiles as r = relu(E1 - F1*G2b); the
    missing rank-1 part F1[j]*G2[i] is restored by per-ACT-tile 1-column
    TF matmuls and one K=1 rank-1 matmul).  gpsimd must NOT touch the
    e-stream: its software tensor_scalar runs ~10x below roofline and it
    shares SBUF ports with DVE, dragging concurrent DVE ops to its speed.
  - The PE main matmuls pipeline at full rate (~209ns/512-col) when fed;
    the s1 projection runs as 64 tiny quadrant matmuls against xT
    (fp32r), ping-ponging two PSUM banks, mostly inside the pre-main DMA
    window.  ACT exps read the s1 PSUM directly.
  - Input DMAs are issued from the scalar/vector/gpsimd queues, which
    clear their framework preamble several us before the sync queue, and
    are sliced so every pipeline stage unblocks just-in-time.  The output
    is written back in 4 slices from 4 different queues as the epilogue
    drains.

Sharding: each core owns N/8 = 1024 query rows i. The host rotates the
j-order per core so the core's own block is always j 0..1023 (all cores
run the identical program; softmax sums are order-invariant).
"""

import sys
import types

import ml_dtypes
import numpy as np

import concourse.bacc as bacc
import concourse.bass as bass
import concourse.mybir as mybir
import concourse.tile as tile
from concourse.bass_utils import run_bass_kernel_spmd


def _install_ntff_hook_shim():
    """The agent image's ``antenv`` lacks ``axon_hooks``; provide it so
    ``run_bass_kernel_spmd(trace=True)`` can capture NTFF profiles."""
    if "antenv.axon_hooks" in sys.modules:
        return
    try:
        from trn_agent_boot.trn_boot import _ntff_profile_via_ctypes

        hook = _ntff_profile_via_ctypes("/opt/axon/libaxon_pjrt.so")
        mod = types.ModuleType("antenv.axon_hooks")
        mod._hook = hook
        mod.get_axon_ntff_profile_hook = lambda: mod._hook
        mod.set_axon_ntff_profile_hook = lambda h: setattr(mod, "_hook", h)
        sys.modules["antenv.axon_hooks"] = mod
    except Exception:
        pass


_install_ntff_hook_shim()

N, D = 8192, 64
NCORES = 8
RB = N // NCORES          # rows (i) per core = 1024
NT = N // 128             # j tiles of 128 = 64
BT = RB // 128            # i tiles per core = 8
DE = D + 1                # x extended with ones column = 65
F32 = mybir.dt.float32
F32R = mybir.dt.float32r
BF16 = mybir.dt.bfloat16
EXP = mybir.ActivationFunctionType.Exp
RELU = mybir.ActivationFunctionType.Relu
COPY = mybir.ActivationFunctionType.Copy
ADD = mybir.AluOpType.add
MUL = mybir.AluOpType.mult
MAX = mybir.AluOpType.max
AX_X = mybir.AxisListType.X
PKW = D + 3 + 128  # packed small-input width (W | b | a1 | a2 | ident)
PKT = PKW + 512    # pack + own-block xT sidecar in one DMA

# s1 chunk order follows input-slab arrival (chunk c = tiles 8c..8c+7):
# c0 from the xo sidecar, then (c4) slab0, (c1,c5) slab1, ...
CHUNKS = [0, 4, 1, 5, 2, 6, 3, 7]
JORDER = list(range(NT))  # mains consume tiles sequentially
# ACT takes 6 e-tiles (in the first 48 so TF closes early); DVE the rest.
_APOS = {6, 13, 20, 27, 34, 41}
EPAT = ["A" if n in _APOS else "V" for n in range(64)]


def build_bass() -> bass.Bass:
    nc = bacc.Bacc(None)
    # xq: x.T quadrant-packed fp32: partitions 0:64 = d rows for j 0..4095
    # (cols = j); partitions 64:128 = d rows for j 4096..8191.
    xq_d = nc.declare_dram_parameter("xq", [128, 4096], F32R, isOutput=False)
    # xo: own-block sidecar (gates G2b + s1 chunk 0 early): partitions
    # 0:64 = d x j 0..511, 64:128 = d x j 512..1023.
    xo_d = nc.declare_dram_parameter("xo", [128, 512], F32R, isOutput=False)
    # xbf: bf16 (x | ones) partition-major: [128, t, de]
    xbf_d = nc.declare_dram_parameter("xbf", [128, NT * DE], BF16, isOutput=False)
    pk_d = nc.declare_dram_parameter("pack", [128, PKW], F32, isOutput=False)
    out_d = nc.declare_dram_parameter("out", [128, BT * D], F32, isOutput=True)

    with tile.TileContext(nc) as tc:
        with (
            tc.tile_pool(name="persist", bufs=1) as persist,
            tc.tile_pool(name="small", bufs=1) as small,
            tc.tile_pool(name="epool", bufs=12) as epool,
            tc.tile_pool(name="opool", bufs=2) as opool,
            tc.tile_pool(name="psumA", bufs=3, space="PSUM") as psumA,
            tc.tile_pool(name="psumS", bufs=1, space="PSUM") as psumS,
            tc.tile_pool(name="psumB", bufs=1, space="PSUM") as psumB,
        ):
            pk = small.tile([128, PKW], F32)
            xo = small.tile([128, 512], F32R)
            xq = persist.tile([128, 4096], F32R)
            xbf_flat = persist.tile([128, NT * DE], BF16)
            x_bf = xbf_flat.rearrange("p (t d) -> p t d", t=NT)

            # ---- input DMAs ----
            # The 16 DMA engines are bandwidth-shared, so transfer order =
            # issue order matters: the tiny prologue-gating pack(+xo) goes
            # first on the scalar queue, then the first x~ slab; the bulk
            # slabs stagger behind on sync's serial issue.
            nc.scalar.dma_start(pk, pk_d[:, :])
            nc.scalar.dma_start(xo, xo_d[:, :])
            nc.scalar.dma_start(xbf_flat[:, 0 : 16 * DE], xbf_d[:, 0 : 16 * DE])
            nc.sync.dma_start(xq[:, 0:1024], xq_d[:, 0:1024])
            nc.sync.dma_start(xq[:, 1024:2048], xq_d[:, 1024:2048])
            nc.sync.dma_start(xq[:, 2048:3072], xq_d[:, 2048:3072])
            nc.sync.dma_start(
                xbf_flat[:, 16 * DE : 32 * DE], xbf_d[:, 16 * DE : 32 * DE]
            )
            nc.sync.dma_start(xq[:, 3072:4096], xq_d[:, 3072:4096])
            nc.sync.dma_start(
                xbf_flat[:, 32 * DE : 48 * DE], xbf_d[:, 32 * DE : 48 * DE]
            )
            nc.sync.dma_start(
                xbf_flat[:, 48 * DE : 64 * DE], xbf_d[:, 48 * DE : 64 * DE]
            )

            W_lo = pk[0:D, 0:D]
            b_lo = pk[0:D, D : D + 1]
            a_lo = pk[0:D, D + 1 : D + 3]
            W_hi = pk[64:128, 0:D]
            a_hi = pk[64:128, D + 1 : D + 3]
            ident = pk[:, D + 3 : D + 3 + 128]

            ones_row = small.tile([1, 128], F32)
            nc.gpsimd.memset(ones_row, 1.0)
            ones_bf = small.tile([1, 128], BF16)
            nc.gpsimd.memset(ones_bf, 1.0)
            # F1 interleaved with zeros (bf16) so TF matmuls have free=2
            F1cb2 = small.tile([128, NT, 2], BF16)
            nc.gpsimd.memset(F1cb2, 0.0)

            # ---------------- tiny projections on PE ----------------
            # v = W.T @ [a1|a2], duplicated into both partition halves so
            # upper-quadrant matmuls have a local rhs.
            v_ps = psumA.tile([128, 2], F32, tag="ps", name="v_ps")
            nc.tensor.matmul(v_ps[0:64, :], lhsT=W_lo, rhs=a_lo, start=True, stop=True)
            nc.tensor.matmul(v_ps[64:128, :], lhsT=W_hi, rhs=a_hi, start=True, stop=True)
            v_r = small.tile([128, 2], F32R)
            nc.vector.tensor_copy(out=v_r, in_=v_ps)

            # c = [b.a1, b.a2] ; c12 = c1+c2 broadcast down 128 partitions
            c_ps = psumA.tile([1, 2], F32, tag="ps", name="c_ps")
            nc.tensor.matmul(c_ps, lhsT=b_lo, rhs=a_lo, start=True, stop=True)
            c_sb = small.tile([1, 2], F32)
            nc.vector.tensor_copy(out=c_sb, in_=c_ps)
            cb_ps = psumA.tile([128, 2], F32, tag="ps", name="cb_ps")
            nc.tensor.matmul(cb_ps, lhsT=ones_row, rhs=c_sb, start=True, stop=True)
            c12 = small.tile([128, 1], F32)
            nc.vector.tensor_reduce(out=c12, in_=cb_ps, axis=AX_X, op=ADD)
            c12s = small.tile([128, 1], F32)
            nc.vector.tensor_scalar(
                out=c12s, in0=c12, scalar1=0.01, scalar2=None, op0=MUL
            )

            # ---------------- p2 of own block -> G2row, G2b ----------------
            G2b = persist.tile([128, RB], BF16)
            g2rows = []
            for h in range(2):
                p2r_ps = psumA.tile([1, 512], F32, tag="ps", name="p2r_ps")
                nc.tensor.matmul(
                    p2r_ps,
                    lhsT=v_r[64 * h : 64 * h + 64, 1:2],
                    rhs=xo[64 * h : 64 * h + 64, :],
                    start=True,
                    stop=True,
                )
                g2row = small.tile([1, 512], BF16, tag=f"g2row{h}", name=f"g2row{h}")
                nc.scalar.activation(out=g2row, in_=p2r_ps, func=EXP, scale=-0.99)
                g2rows.append(g2row)
                gb_ps = psumA.tile([128, 512], F32, tag="ps", name="gb_ps")
                nc.tensor.matmul(
                    gb_ps, lhsT=ones_bf, rhs=g2row, start=True, stop=True
                )
                nc.vector.tensor_copy(
                    out=G2b[:, h * 512 : (h + 1) * 512], in_=gb_ps
                )

            # ---------------- s1 on PE + exps on ACT ----------------
            # s1 tile t: lhsT = [64,128] quadrant slice of xT, rhs = v
            # -> psum [128, 2] (col 0 = p1).  Within a chunk the 8 matmuls
            # ping-pong two PSUM banks so they pipeline; ACT exps read the
            # PSUM halves directly (strided writes into E1c/F1c).
            E1c = small.tile([128, NT], F32)
            F1c = small.tile([128, NT], F32)
            nF1c = small.tile([128, NT], F32)
            for c in CHUNKS:
                halves = [
                    psumS.tile([128, 4, 2], F32, tag="s1a", name=f"s1a{c}"),
                    psumS.tile([128, 4, 2], F32, tag="s1b", name=f"s1b{c}"),
                ]
                for k in range(8):
                    t = 8 * c + k
                    if c == 0:
                        lhsT = xo[64 * (t // 4) : 64 * (t // 4) + 64,
                                  128 * (t % 4) : 128 * (t % 4) + 128]
                        rhs = v_r[64 * (t // 4) : 64 * (t // 4) + 64, 0:2]
                    elif t < 32:
                        lhsT = xq[0:64, 128 * t : 128 * (t + 1)]
                        rhs = v_r[0:64, 0:2]
                    else:
                        lhsT = xq[64:128, 128 * (t - 32) : 128 * (t - 31)]
                        rhs = v_r[64:128, 0:2]
                    nc.tensor.matmul(
                        halves[k % 2][:, k // 2, :],
                        lhsT=lhsT,
                        rhs=rhs,
                        start=True,
                        stop=True,
                        skip_group_check=True,
                    )
                for par in range(2):
                    cs = slice(8 * c + par, 8 * c + 8, 2)
                    p1col = halves[par][:, :, 0:1]
                    nc.scalar.activation(
                        out=E1c[:, cs], in_=p1col, func=EXP, bias=c12, scale=1.0
                    )
                    nc.scalar.activation(
                        out=F1c[:, cs], in_=p1col, func=EXP, bias=c12s, scale=0.01
                    )
                cs = slice(8 * c, 8 * (c + 1))
                nc.scalar.copy(out=F1cb2[:, cs, 0:1], in_=F1c[:, cs])
                nc.vector.tensor_scalar(
                    out=nF1c[:, cs], in0=F1c[:, cs], scalar1=-1.0, scalar2=None,
                    op0=MUL,
                )

            # perf probes in the pre-main DVE idle window: measure whether
            # 1-ptr-scalar and immediate-scalar tensor_scalar variants
            # reach the 4x DVE rate (the 2-ptr mult+max op measures 2x).
            # Results are read from the trace; outputs are unused scratch.
            probe = small.tile([128, RB], BF16)
            nc.vector.tensor_scalar(
                out=probe, in0=G2b, scalar1=E1c[:, 0:1], scalar2=None, op0=MAX
            )
            nc.vector.tensor_scalar(
                out=probe, in0=G2b, scalar1=2.0, scalar2=None, op0=MUL
            )

            # ---------------- main loop ----------------
            acc0 = psumB.tile([128, 512], F32, tag="acc0", name="acc0")
            acc1 = psumB.tile([128, 512], F32, tag="acc1", name="acc1")
            accs = [acc0, acc1]
            tf_ps = psumB.tile([128, 2], F32, tag="tf", name="tf_ps")
            act_jts = [jt for n, jt in enumerate(JORDER) if EPAT[n] == "A"]
            first_act, last_act = act_jts[0], act_jts[-1]

            for n, jt in enumerate(JORDER):
                eng = EPAT[n]
                e_t = epool.tile([128, RB], BF16, tag="e", name="e_t")
                if eng == "V":
                    nc.vector.tensor_scalar(
                        out=e_t, in0=G2b,
                        scalar1=F1c[:, jt : jt + 1], scalar2=E1c[:, jt : jt + 1],
                        op0=MUL, op1=MAX,
                    )
                else:  # ACT: relu(E1 - F1*G2b); rank-1 part restored later
                    nc.scalar.activation(
                        out=e_t, in_=G2b, func=RELU,
                        scale=nF1c[:, jt : jt + 1], bias=E1c[:, jt : jt + 1],
                    )
                lhsT = x_bf[:, jt, 0:DE]
                for h in range(2):
                    nc.tensor.matmul(
                        accs[h][0:DE, :],
                        lhsT=lhsT,
                        rhs=e_t[:, h * 512 : (h + 1) * 512],
                        start=(n == 0),
                        stop=False,
                    )
                if eng == "A":
                    nc.tensor.matmul(
                        tf_ps[0:DE, :],
                        lhsT=lhsT,
                        rhs=F1cb2[:, jt, :],
                        start=(jt == first_act),
                        stop=(jt == last_act),
                    )

            # rank-1 completion: accs[h] += TF_act[d] * G2row[i]
            tf_sb = small.tile([DE, 1], F32)
            nc.scalar.copy(out=tf_sb, in_=tf_ps[0:DE, 0:1])
            tfT_ps = psumA.tile([1, DE], F32, tag="ps", name="tfT_ps")
            nc.tensor.transpose(tfT_ps, tf_sb, ident[:DE, :DE])
            tf_row = small.tile([1, DE], BF16)
            nc.scalar.copy(out=tf_row, in_=tfT_ps)
            for h in range(2):
                nc.tensor.matmul(
                    accs[h][0:DE, :],
                    lhsT=tf_row,
                    rhs=g2rows[h],
                    start=False,
                    stop=True,
                )

            # ---------------- epilogue: normalize + store ----------------
            # per t-tile: PE transpose -> DVE reciprocal -> ACT normalize;
            # output leaves in 4 slices from 4 different DMA queues.
            outT = small.tile([DE, RB], F32)
            for h in range(2):
                nc.scalar.copy(
                    out=outT[:, h * 512 : (h + 1) * 512],
                    in_=accs[h][0:DE, :],
                )
            out_flat = small.tile([128, BT * D], F32)
            out_sb = out_flat.rearrange("p (t d) -> p t d", t=BT)
            oq = [nc.sync, nc.scalar, nc.gpsimd, nc.scalar]
            for t in range(BT):
                tp2 = psumA.tile([128, DE], F32, tag="ps", name="tp2")
                nc.tensor.transpose(
                    tp2, outT[:, t * 128 : (t + 1) * 128], ident[:DE, :DE]
                )
                rcol = opool.tile([128, 1], F32, tag="rcol", name="rcol")
                nc.vector.reciprocal(rcol, tp2[:, D : D + 1])
                nc.scalar.activation(
                    out=out_sb[:, t, :], in_=tp2[:, 0:D], func=COPY, scale=rcol
                )
                if t % 2 == 1:
                    q = oq[t // 2]
                    q.dma_start(
                        out_d[:, (t - 1) * D : (t + 1) * D],
                        out_flat[:, (t - 1) * D : (t + 1) * D],
                    )

    nc.finalize()
    return nc


def _execute(inputs: dict, trace: bool = False):
    x = np.ascontiguousarray(np.asarray(inputs["x"], dtype=np.float32))
    W = np.ascontiguousarray(np.asarray(inputs["W"], dtype=np.float32))
    b = np.asarray(inputs["b"], dtype=np.float32).reshape(D)
    a = np.asarray(inputs["a"], dtype=np.float32).reshape(2 * D)
    assert x.shape == (N, D) and W.shape == (D, D)

    pack0 = np.zeros((128, PKW), np.float32)
    for half in (slice(0, 64), slice(64, 128)):
        pack0[half, 0:D] = W
        pack0[half, D] = b
        pack0[half, D + 1] = a[:D]
        pack0[half, D + 2] = a[D:]
    pack0[:, D + 3 : D + 3 + 128] = np.eye(128, dtype=np.float32)

    xe = np.concatenate([x, np.ones((N, 1), np.float32)], axis=1)  # [N, 65]

    nc = build_bass()
    in_maps = []
    for c in range(NCORES):
        # rotate j-order so this core's own block is j 0..1023
        perm = np.concatenate(
            [np.arange(c * RB, (c + 1) * RB), np.arange(0, c * RB),
             np.arange((c + 1) * RB, N)]
        )
        xT = x[perm].T          # [D, N] rotated
        xq = np.ascontiguousarray(
            np.concatenate([xT[:, 0:4096], xT[:, 4096:8192]], axis=0)
        )
        xo = np.ascontiguousarray(
            np.concatenate([xT[:, 0:512], xT[:, 512:1024]], axis=0)
        )
        xbf = np.ascontiguousarray(
            xe[perm].reshape(NT, 128, DE)
            .transpose(1, 0, 2)
            .reshape(128, NT * DE)
            .astype(ml_dtypes.bfloat16)
        )
        in_maps.append({"xq": xq, "xo": xo, "xbf": xbf, "pack": pack0})
    res = run_bass_kernel_spmd(
        nc, in_maps, core_ids=list(range(NCORES)), trace=trace
    )
    # un-permute each core's output: (p, t*D+d) -> (t*128+p, d)
    outs = []
    for r in res.results:
        o = r["out"].reshape(128, BT, D).transpose(1, 0, 2).reshape(RB, D)
        outs.append(o)
    out = np.ascontiguousarray(np.concatenate(outs, axis=0))
    return out, res


def kernel(x, W, b, a):
    out, _ = _execute({"x": x, "W": W, "b": b, "a": a})
    return out


# revision 23
# speedup vs baseline: 1.2357x; 1.0788x over previous
"""GAT layer (N=8192, D=64) as a Bass/Tile kernel on 8 TRN2 NeuronCores.

Math (reference):
    h  = x @ W.T + b
    s1 = h @ a1 ; s2 = h @ a2                    # [N] each
    score[i,j] = s2[i] + s1[j]
    att = softmax_j(leaky_relu(score))
    out = att @ x

Reformulation:
    Fold the linear layer:  v = W.T @ [a1|a2], c_k = b.a_k
      p1 = x @ v1 ; p2 = x @ v2 ; s1 = p1 + c1 ; s2 = p2 + c2
    Softmax rows are shift invariant, so subtract p2[i] from row i. With
    per-j scalars E1 = exp(sh1), F1 = exp(0.01*sh1) (sh1 = p1 + c1 + c2)
    and the broadcast tile G2b[j,i] = exp(-0.99*p2[i]):
      e[j,i] = max( G2b[j,i] * F1[j],  E1[j] )
    The final matmul (ones-column appended to x for the softmax
    denominator) accumulates over j in PSUM:
      outT[0:65, i] += x_ext[j,:].T @ e[j,i] ; Z[i] = outT[64,i]

Schedule (informed by HW traces):
  - e-tiles are produced on DVE (58 tiles, tensor_scalar mult+max at the
    2x bf16 rate, ~584ns) and ACT (6 tiles as r = relu(E1 - F1*G2b); the
    missing rank-1 part F1[j]*G2[i] is restored by per-ACT-tile 1-column
    TF matmuls and one K=1 rank-1 matmul).  gpsimd must NOT touch the
    e-stream: its software tensor_scalar runs ~10x below roofline and it
    shares SBUF ports with DVE, dragging concurrent DVE ops to its speed.
  - The PE main matmuls pipeline at full rate (~209ns/512-col) when fed;
    the s1 projection runs as 64 tiny quadrant matmuls against xT
    (fp32r), ping-ponging two PSUM banks, mostly inside the pre-main DMA
    window.  ACT exps read the s1 PSUM directly.
  - Input DMAs are issued from the scalar/vector/gpsimd queues, which
    clear their framework preamble several us before the sync queue, and
    are sliced so every pipeline stage unblocks just-in-time.  The output
    is written back in 4 slices from 4 different queues as the epilogue
    drains.

Sharding: each core owns N/8 = 1024 query rows i. The host rotates the
j-order per core so the core's own block is always j 0..1023 (all cores
run the identical program; softmax sums are order-invariant).
"""

import sys
import types

import ml_dtypes
import numpy as np

import concourse.bacc as bacc
import concourse.bass as bass
import concourse.mybir as mybir
import concourse.tile as tile
from concourse.bass_utils import run_bass_kernel_spmd


def _install_ntff_hook_shim():
    """The agent image's ``antenv`` lacks ``axon_hooks``; provide it so
    ``run_bass_kernel_spmd(trace=True)`` can capture NTFF profiles."""
    if "antenv.axon_hooks" in sys.modules:
        return
    try:
        from trn_agent_boot.trn_boot import _ntff_profile_via_ctypes

        hook = _ntff_profile_via_ctypes("/opt/axon/libaxon_pjrt.so")
        mod = types.ModuleType("antenv.axon_hooks")
        mod._hook = hook
        mod.get_axon_ntff_profile_hook = lambda: mod._hook
        mod.set_axon_ntff_profile_hook = lambda h: setattr(mod, "_hook", h)
        sys.modules["antenv.axon_hooks"] = mod
    except Exception:
        pass


_install_ntff_hook_shim()

N, D = 8192, 64
NCORES = 8
RB = N // NCORES          # rows (i) per core = 1024
NT = N // 128             # j tiles of 128 = 64
BT = RB // 128            # i tiles per core = 8
DE = D + 1                # x extended with ones column = 65
F32 = mybir.dt.float32
F32R = mybir.dt.float32r
BF16 = mybir.dt.bfloat16
EXP = mybir.ActivationFunctionType.Exp
RELU = mybir.ActivationFunctionType.Relu
COPY = mybir.ActivationFunctionType.Copy
ADD = mybir.AluOpType.add
MUL = mybir.AluOpType.mult
MAX = mybir.AluOpType.max
AX_X = mybir.AxisListType.X
PKW = D + 3 + 128  # packed small-input width (W | b | a1 | a2 | ident)
PKT = PKW + 512    # pack + own-block xT sidecar in one DMA

# s1 chunk order follows input-slab arrival (chunk c = tiles 8c..8c+7):
# c0 from the xo sidecar, then (c4) slab0, (c1,c5) slab1, ...
CHUNKS = [0, 4, 1, 5, 2, 6, 3, 7]
JORDER = list(range(NT))  # mains consume tiles sequentially
# ACT takes 6 e-tiles (in the first 48 so TF closes early); DVE the rest.
_APOS = {6, 13, 20, 27, 34, 41}
EPAT = ["A" if n in _APOS else "V" for n in range(64)]


def build_bass() -> bass.Bass:
    nc = bacc.Bacc(None)
    # xp: x partition-major fp32 [128, t, d] (feeds the gpsimd s1 path)
    xp_d = nc.declare_dram_parameter("xp", [128, NT * D], F32, isOutput=False)
    # xo: own-block sidecar (gates G2b + s1 chunk 0 early): partitions
    # 0:64 = d x j 0..511, 64:128 = d x j 512..1023.
    xo_d = nc.declare_dram_parameter("xo", [128, 512], F32R, isOutput=False)
    # xbf: bf16 (x | ones) partition-major: [128, t, de]
    xbf_d = nc.declare_dram_parameter("xbf", [128, NT * DE], BF16, isOutput=False)
    pk_d = nc.declare_dram_parameter("pack", [128, PKW], F32, isOutput=False)
    out_d = nc.declare_dram_parameter("out", [128, BT * D], F32, isOutput=True)

    with tile.TileContext(nc) as tc:
        with (
            tc.tile_pool(name="persist", bufs=1) as persist,
            tc.tile_pool(name="small", bufs=1) as small,
            tc.tile_pool(name="epool", bufs=12) as epool,
            tc.tile_pool(name="work", bufs=2) as work,
            tc.tile_pool(name="opool", bufs=2) as opool,
            tc.tile_pool(name="psumA", bufs=3, space="PSUM") as psumA,
            tc.tile_pool(name="psumS", bufs=1, space="PSUM") as psumS,
            tc.tile_pool(name="psumB", bufs=1, space="PSUM") as psumB,
        ):
            pk = small.tile([128, PKW], F32)
            xo = small.tile([128, 512], F32R)
            xp_flat = persist.tile([128, NT * D], F32)
            x_sb = xp_flat.rearrange("p (t d) -> p t d", t=NT)
            xbf_flat = persist.tile([128, NT * DE], BF16)
            x_bf = xbf_flat.rearrange("p (t d) -> p t d", t=NT)

            # ---- input DMAs ----
            # The 16 DMA engines are bandwidth-shared, so transfer order =
            # issue order matters: the tiny prologue-gating pack(+xo) goes
            # first on the scalar queue, then the first x~ slab; the bulk
            # slabs stagger behind on sync's serial issue.
            nc.scalar.dma_start(pk, pk_d[:, :])
            nc.scalar.dma_start(xo, xo_d[:, :])
            nc.scalar.dma_start(xbf_flat[:, 0 : 16 * DE], xbf_d[:, 0 : 16 * DE])
            QD = 16 * D
            nc.sync.dma_start(xp_flat[:, 0:QD], xp_d[:, 0:QD])
            nc.sync.dma_start(xp_flat[:, QD : 2 * QD], xp_d[:, QD : 2 * QD])
            nc.sync.dma_start(
                xbf_flat[:, 16 * DE : 32 * DE], xbf_d[:, 16 * DE : 32 * DE]
            )
            nc.sync.dma_start(xp_flat[:, 2 * QD : 3 * QD], xp_d[:, 2 * QD : 3 * QD])
            nc.sync.dma_start(
                xbf_flat[:, 32 * DE : 48 * DE], xbf_d[:, 32 * DE : 48 * DE]
            )
            nc.sync.dma_start(xp_flat[:, 3 * QD : 4 * QD], xp_d[:, 3 * QD : 4 * QD])
            nc.sync.dma_start(
                xbf_flat[:, 48 * DE : 64 * DE], xbf_d[:, 48 * DE : 64 * DE]
            )

            W_lo = pk[0:D, 0:D]
            b_lo = pk[0:D, D : D + 1]
            a_lo = pk[0:D, D + 1 : D + 3]
            W_hi = pk[64:128, 0:D]
            a_hi = pk[64:128, D + 1 : D + 3]
            ident = pk[:, D + 3 : D + 3 + 128]

            ones_row = small.tile([1, 128], F32)
            nc.gpsimd.memset(ones_row, 1.0)
            ones_bf = small.tile([1, 128], BF16)
            nc.gpsimd.memset(ones_bf, 1.0)
            # F1 interleaved with zeros (bf16) so TF matmuls have free=2
            F1cb2 = small.tile([128, NT, 2], BF16)
            nc.gpsimd.memset(F1cb2, 0.0)

            # ---------------- tiny projections on PE ----------------
            # v = W.T @ [a1|a2], duplicated into both partition halves so
            # upper-quadrant matmuls have a local rhs.
            v_ps = psumA.tile([128, 2], F32, tag="ps", name="v_ps")
            nc.tensor.matmul(v_ps[0:64, :], lhsT=W_lo, rhs=a_lo, start=True, stop=True)
            nc.tensor.matmul(v_ps[64:128, :], lhsT=W_hi, rhs=a_hi, start=True, stop=True)
            v_r = small.tile([128, 2], F32R)
            nc.vector.tensor_copy(out=v_r, in_=v_ps)
            v_sb = small.tile([128, 2], F32)
            nc.vector.tensor_copy(out=v_sb, in_=v_ps)

            # c = [b.a1, b.a2] ; c12 = c1+c2 broadcast down 128 partitions
            c_ps = psumA.tile([1, 2], F32, tag="ps", name="c_ps")
            nc.tensor.matmul(c_ps, lhsT=b_lo, rhs=a_lo, start=True, stop=True)
            c_sb = small.tile([1, 2], F32)
            nc.vector.tensor_copy(out=c_sb, in_=c_ps)
            cb_ps = psumA.tile([128, 2], F32, tag="ps", name="cb_ps")
            nc.tensor.matmul(cb_ps, lhsT=ones_row, rhs=c_sb, start=True, stop=True)
            c12 = small.tile([128, 1], F32)
            nc.vector.tensor_reduce(out=c12, in_=cb_ps, axis=AX_X, op=ADD)
            c12s = small.tile([128, 1], F32)
            nc.vector.tensor_scalar(
                out=c12s, in0=c12, scalar1=0.01, scalar2=None, op0=MUL
            )

            # ---------------- p2 of own block -> G2row, G2b ----------------
            G2b = persist.tile([128, RB], BF16)
            g2rows = []
            for h in range(2):
                p2r_ps = psumA.tile([1, 512], F32, tag="ps", name="p2r_ps")
                nc.tensor.matmul(
                    p2r_ps,
                    lhsT=v_r[64 * h : 64 * h + 64, 1:2],
                    rhs=xo[64 * h : 64 * h + 64, :],
                    start=True,
                    stop=True,
                )
                g2row = small.tile([1, 512], BF16, tag=f"g2row{h}", name=f"g2row{h}")
                nc.scalar.activation(out=g2row, in_=p2r_ps, func=EXP, scale=-0.99)
                g2rows.append(g2row)
                gb_ps = psumA.tile([128, 512], F32, tag="ps", name="gb_ps")
                nc.tensor.matmul(
                    gb_ps, lhsT=ones_bf, rhs=g2row, start=True, stop=True
                )
                nc.vector.tensor_copy(
                    out=G2b[:, h * 512 : (h + 1) * 512], in_=gb_ps
                )

            # ---------------- s1: chunk 0 on PE (from xo), rest on gpsimd ----
            # v1 broadcast [128, D] for the gpsimd multiply
            v1r_ps = psumA.tile([1, D], F32, tag="ps", name="v1r_ps")
            nc.tensor.transpose(v1r_ps, v_sb[0:64, 0:1], ident[:D, :D])
            v1row = small.tile([1, D], F32)
            nc.vector.tensor_copy(out=v1row, in_=v1r_ps)
            v1b_ps = psumA.tile([128, D], F32, tag="ps", name="v1b_ps")
            nc.tensor.matmul(
                v1b_ps, lhsT=ones_row, rhs=v1row, start=True, stop=True
            )
            v1b = small.tile([128, D], F32)
            nc.vector.tensor_copy(out=v1b, in_=v1b_ps)
            v1b_b = bass.AP(
                tensor=v1b.tensor,
                offset=v1b.offset,
                ap=[v1b.ap[0], [0, 8], v1b.ap[1]],
            )

            E1c = small.tile([128, NT], F32)
            F1c = small.tile([128, NT], F32)
            nF1c = small.tile([128, NT], F32)
            s1c = small.tile([128, NT], F32)

            def _post_chunk(c):
                cs = slice(8 * c, 8 * (c + 1))
                nc.scalar.copy(out=F1cb2[:, cs, 0:1], in_=F1c[:, cs])
                nc.gpsimd.tensor_scalar(
                    out=nF1c[:, cs], in0=F1c[:, cs], scalar1=-1.0, scalar2=None,
                    op0=MUL,
                )

            # chunk 0: 8 tiny quadrant matmuls against xo (PE is idle here)
            halves = [
                psumS.tile([128, 4, 2], F32, tag="s1a", name="s1a0"),
                psumS.tile([128, 4, 2], F32, tag="s1b", name="s1b0"),
            ]
            for k in range(8):
                nc.tensor.matmul(
                    halves[k % 2][:, k // 2, :],
                    lhsT=xo[64 * (k // 4) : 64 * (k // 4) + 64,
                            128 * (k % 4) : 128 * (k % 4) + 128],
                    rhs=v_r[64 * (k // 4) : 64 * (k // 4) + 64, 0:2],
                    start=True,
                    stop=True,
                    skip_group_check=True,
                )
            for par in range(2):
                cs = slice(par, 8, 2)
                p1col = halves[par][:, :, 0:1]
                nc.scalar.activation(
                    out=E1c[:, cs], in_=p1col, func=EXP, bias=c12, scale=1.0
                )
                nc.scalar.activation(
                    out=F1c[:, cs], in_=p1col, func=EXP, bias=c12s, scale=0.01
                )
            _post_chunk(0)

            # chunks 1..7: gpsimd multiply + reduce (fp32 gpsimd ops coexist
            # with DVE at full speed; bf16 gpsimd ops must stay banned)
            for c in range(1, 8):
                cs = slice(8 * c, 8 * (c + 1))
                tmp = work.tile([128, 8, D], F32, tag="tmp", name="tmp")
                nc.gpsimd.tensor_mul(tmp, x_sb[:, 8 * c : 8 * (c + 1), :], v1b_b)
                nc.vector.tensor_reduce(
                    out=s1c[:, cs], in_=tmp, axis=AX_X, op=ADD
                )
                nc.scalar.activation(
                    out=E1c[:, cs], in_=s1c[:, cs], func=EXP, bias=c12, scale=1.0
                )
                nc.scalar.activation(
                    out=F1c[:, cs], in_=s1c[:, cs], func=EXP, bias=c12s,
                    scale=0.01,
                )
                _post_chunk(c)

            # ---------------- main loop ----------------
            acc0 = psumB.tile([128, 512], F32, tag="acc0", name="acc0")
            acc1 = psumB.tile([128, 512], F32, tag="acc1", name="acc1")
            accs = [acc0, acc1]
            tf_ps = psumB.tile([128, 2], F32, tag="tf", name="tf_ps")
            act_jts = [jt for n, jt in enumerate(JORDER) if EPAT[n] == "A"]
            first_act, last_act = act_jts[0], act_jts[-1]

            for n, jt in enumerate(JORDER):
                eng = EPAT[n]
                e_t = epool.tile([128, RB], BF16, tag="e", name="e_t")
                if eng == "V":
                    nc.vector.tensor_scalar(
                        out=e_t, in0=G2b,
                        scalar1=F1c[:, jt : jt + 1], scalar2=E1c[:, jt : jt + 1],
                        op0=MUL, op1=MAX,
                    )
                else:  # ACT: relu(E1 - F1*G2b); rank-1 part restored later
                    nc.scalar.activation(
                        out=e_t, in_=G2b, func=RELU,
                        scale=nF1c[:, jt : jt + 1], bias=E1c[:, jt : jt + 1],
                    )
                lhsT = x_bf[:, jt, 0:DE]
                for h in range(2):
                    nc.tensor.matmul(
                        accs[h][0:DE, :],
                        lhsT=lhsT,
                        rhs=e_t[:, h * 512 : (h + 1) * 512],
                        start=(n == 0),
                        stop=False,
                    )
                if eng == "A":
                    nc.tensor.matmul(
                        tf_ps[0:DE, :],
                        lhsT=lhsT,
                        rhs=F1cb2[:, jt, :],
                        start=(jt == first_act),
                        stop=(jt == last_act),
                    )

            # rank-1 completion: accs[h] += TF_act[d] * G2row[i]
            tf_sb = small.tile([DE, 1], F32)
            nc.scalar.copy(out=tf_sb, in_=tf_ps[0:DE, 0:1])
            tfT_ps = psumA.tile([1, DE], F32, tag="ps", name="tfT_ps")
            nc.tensor.transpose(tfT_ps, tf_sb, ident[:DE, :DE])
            tf_row = small.tile([1, DE], BF16)
            nc.scalar.copy(out=tf_row, in_=tfT_ps)
            for h in range(2):
                nc.tensor.matmul(
                    accs[h][0:DE, :],
                    lhsT=tf_row,
                    rhs=g2rows[h],
                    start=False,
                    stop=True,
                )

            # ---------------- epilogue: normalize + store ----------------
            # per t-tile: PE transpose -> DVE reciprocal -> ACT normalize;
            # output leaves in 4 slices from 4 different DMA queues.
            outT = small.tile([DE, RB], F32)
            for h in range(2):
                nc.scalar.copy(
                    out=outT[:, h * 512 : (h + 1) * 512],
                    in_=accs[h][0:DE, :],
                )
            out_flat = small.tile([128, BT * D], F32)
            out_sb = out_flat.rearrange("p (t d) -> p t d", t=BT)
            oq = [nc.sync, nc.scalar, nc.gpsimd, nc.scalar]
            for t in range(BT):
                tp2 = psumA.tile([128, DE], F32, tag="ps", name="tp2")
                nc.tensor.transpose(
                    tp2, outT[:, t * 128 : (t + 1) * 128], ident[:DE, :DE]
                )
                rcol = opool.tile([128, 1], F32, tag="rcol", name="rcol")
                nc.vector.reciprocal(rcol, tp2[:, D : D + 1])
                nc.scalar.activation(
                    out=out_sb[:, t, :], in_=tp2[:, 0:D], func=COPY, scale=rcol
                )
                if t % 2 == 1:
                    q = oq[t // 2]
                    q.dma_start(
                        out_d[:, (t - 1) * D : (t + 1) * D],
                        out_flat[:, (t - 1) * D : (t + 1) * D],
                    )

    nc.finalize()
    return nc


def _execute(inputs: dict, trace: bool = False):
    x = np.ascontiguousarray(np.asarray(inputs["x"], dtype=np.float32))
    W = np.ascontiguousarray(np.asarray(inputs["W"], dtype=np.float32))
    b = np.asarray(inputs["b"], dtype=np.float32).reshape(D)
    a = np.asarray(inputs["a"], dtype=np.float32).reshape(2 * D)
    assert x.shape == (N, D) and W.shape == (D, D)

    pack0 = np.zeros((128, PKW), np.float32)
    for half in (slice(0, 64), slice(64, 128)):
        pack0[half, 0:D] = W
        pack0[half, D] = b
        pack0[half, D + 1] = a[:D]
        pack0[half, D + 2] = a[D:]
    pack0[:, D + 3 : D + 3 + 128] = np.eye(128, dtype=np.float32)

    xe = np.concatenate([x, np.ones((N, 1), np.float32)], axis=1)  # [N, 65]

    nc = build_bass()
    in_maps = []
    for c in range(NCORES):
        # rotate j-order so this core's own block is j 0..1023
        perm = np.concatenate(
            [np.arange(c * RB, (c + 1) * RB), np.arange(0, c * RB),
             np.arange((c + 1) * RB, N)]
        )
        xpm = x[perm]           # [N, D] rotated
        xT = xpm.T              # [D, N]
        xp = np.ascontiguousarray(
            xpm.reshape(NT, 128, D).transpose(1, 0, 2).reshape(128, NT * D)
        )
        xo = np.ascontiguousarray(
            np.concatenate([xT[:, 0:512], xT[:, 512:1024]], axis=0)
        )
        xbf = np.ascontiguousarray(
            xe[perm].reshape(NT, 128, DE)
            .transpose(1, 0, 2)
            .reshape(128, NT * DE)
            .astype(ml_dtypes.bfloat16)
        )
        in_maps.append({"xp": xp, "xo": xo, "xbf": xbf, "pack": pack0})
    res = run_bass_kernel_spmd(
        nc, in_maps, core_ids=list(range(NCORES)), trace=trace
    )
    # un-permute each core's output: (p, t*D+d) -> (t*128+p, d)
    outs = []
    for r in res.results:
        o = r["out"].reshape(128, BT, D).transpose(1, 0, 2).reshape(RB, D)
        outs.append(o)
    out = np.ascontiguousarray(np.concatenate(outs, axis=0))
    return out, res


def kernel(x, W, b, a):
    out, _ = _execute({"x": x, "W": W, "b": b, "a": a})
    return out
